# revision 9
# baseline (speedup 1.0000x reference)
"""Trainium2 Bass kernel for nn_PixelTwoStageOrdinalHead.

Self-contained: shards the batch (B=8) across 8 NeuronCores (one sample per
core), runs two conv heads (Conv3x3 -> GroupNorm(8) -> GELU -> Conv1x1), the
CORN decode, and the per-sample masked mean / top-k aggregation fully on
device, then reassembles full-shape outputs on host.
"""
import sys, os
sys.path.insert(0, '/opt/trn_rl_repo')
import numpy as np
from contextlib import ExitStack

import concourse.bass as bass
import concourse.tile as tile
from concourse import mybir, bacc
from concourse.bass_utils import run_bass_kernel_spmd

F32 = mybir.dt.float32
F32R = mybir.dt.float32r
I32 = mybir.dt.int32
OP = mybir.AluOpType
AF = mybir.ActivationFunctionType
AX = mybir.AxisListType

HW = 128            # H = W
NPIX = HW * HW      # 16384
CI = 256
CH = 256
GROUPS = 8
EPS = 1e-5
TOPK_RATIO = 0.2
MEAN_W, TOPK_W = 0.7, 0.3
NCORES = 8
PADW = 130          # padded row width
BSEARCH_ITERS = 26

_cache = {}


def _emit(nc, tc, io):
    ctx = ExitStack()
    P = 128
    # ---------------- pools ----------------
    cpool = ctx.enter_context(tc.tile_pool(name="consts", bufs=1))
    xpool = ctx.enter_context(tc.tile_pool(name="xpad", bufs=1))
    wpool = ctx.enter_context(tc.tile_pool(name="w1", bufs=2))
    hpool = ctx.enter_context(tc.tile_pool(name="hev", bufs=4))
    p2pool = ctx.enter_context(tc.tile_pool(name="p2", bufs=3))
    gapool = ctx.enter_context(tc.tile_pool(name="ga", bufs=3))
    lgpool = ctx.enter_context(tc.tile_pool(name="lg", bufs=2))
    dpool = ctx.enter_context(tc.tile_pool(name="dec", bufs=1))
    spool = ctx.enter_context(tc.tile_pool(name="small", bufs=1))
    scpool = ctx.enter_context(tc.tile_pool(name="scr", bufs=1))
    psc = ctx.enter_context(tc.tile_pool(name="psc", bufs=4, space="PSUM"))
    ps1 = ctx.enter_context(tc.tile_pool(name="ps1", bufs=2, space="PSUM"))
    pss = ctx.enter_context(tc.tile_pool(name="pss", bufs=2, space="PSUM"))

    # ---------------- consts / weights ----------------
    tcst = cpool.tile([P, 133], F32)
    nc.sync.dma_start(tcst[:], io['cst'])
    blockones = tcst[:, 0:128]        # [128,128] 32-block diag ones
    ones_col = tcst[:, 128:129]       # [128,1] ones
    onehot4 = tcst[:, 129:133]        # [128,4] p//32 onehot
    tcrow = cpool.tile([1, 136], F32)
    nc.sync.dma_start(tcrow[:], io['crow'])
    ones_row = tcrow[:, 0:128]        # [1,128] ones
    iota4 = tcrow[:, 128:132]         # [1,4] 0..3
    tcb = cpool.tile([4, 128], F32)   # bcast4: [4,128] group->channel
    nc.sync.dma_start(tcb[:], io['cstb'])

    # gn params / w2 / b2
    gnt = []
    w2t = []
    b2t = []
    for h in range(2):
        g = []
        for c in range(2):
            t = cpool.tile([P, 2], F32, tag=f"gn{h}{c}")
            nc.sync.dma_start(t[:], io['gn'][h, c])
            g.append(t)
        gnt.append(g)
        w = []
        for c in range(2):
            t = cpool.tile([P, 2], F32R, tag=f"w2{h}{c}")
            nc.sync.dma_start(t[:], io['w2'][h, c].bitcast(F32R))
            w.append(t)
        w2t.append(w)
        t = cpool.tile([2, 1], F32, tag=f"b2{h}")
        nc.sync.dma_start(t[:], io['b2'][h])
        b2t.append(t)

    # ---------------- x padded ----------------
    xp = []
    for c in range(2):
        t = xpool.tile([P, PADW * PADW], F32R, tag=f"xp{c}")
        t3 = t[:].rearrange("p (r q) -> p r q", q=PADW)
        zz = io['zeros'].bitcast(F32R)
        nc.sync.dma_start(t3[:, 0, :], zz)
        nc.sync.dma_start(t3[:, PADW - 1, :], zz)
        nc.sync.dma_start(t3[:, :, 0], zz)
        nc.sync.dma_start(t3[:, :, PADW - 1], zz)
        xin = io['x'][c * P:(c + 1) * P, :].rearrange("p (r q) -> p r q", q=HW)
        nc.sync.dma_start(t3[:, 1:1 + HW, 1:1 + HW], xin.bitcast(F32R))
        xp.append(t3)

    KSTAGE = int(os.environ.get("KSTAGE", "4"))
    co_heads = [1, 2]   # conv1x1 out channels per head
    AB = [[None, None], [None, None]]   # per head, per chunk: (A,B) [128,1]

    # ============ per-head conv3x3 + stats ============
    sumcols = []
    for h in range(2):
        w1 = []
        for c in range(2):
            t = wpool.tile([P, 9 * 256], F32R, tag="w1")
            nc.sync.dma_start(t[:], io['w1'][h, c].bitcast(F32R))
            w1.append(t)
        sc = []
        for c in range(2):
            s = spool.tile([P, 64], F32, tag=f"stats{h}{c}")
            sc.append(s)
        sumcols.append(sc)

        for c_o in range(2):
            for sg in range(8):
                pts = []
                for b in range(4):
                    pts.append(psc.tile([P, 512], F32, tag="convps", name="convps"))
                for ci in range(2):
                    for tap in range(9):
                        dy, dx = tap // 3 - 1, tap % 3 - 1
                        lhsT = w1[ci][:, tap * 256 + c_o * P: tap * 256 + c_o * P + P]
                        for b in range(4):
                            q0 = sg * 2048 + b * 512
                            r0 = q0 // HW
                            rhs = xp[ci][:, r0 + 1 + dy: r0 + 5 + dy, 1 + dx: 129 + dx]
                            nc.tensor.matmul(
                                pts[b][:], lhsT, rhs,
                                start=(ci == 0 and tap == 0),
                                stop=(ci == 1 and tap == 8))
                for b in range(4):
                    q0 = sg * 2048 + b * 512
                    ti = sg * 4 + b
                    ht = hpool.tile([P, 512], F32, tag="hev")
                    nc.vector.tensor_scalar(
                        ht[:], pts[b][:], 0.0, 0.0, op0=OP.add, op1=OP.add,
                        accum_out=sumcols[h][c_o][:, ti:ti + 1])
                    scr = scpool.tile([P, 512], F32, tag="scr")
                    nc.vector.scalar_tensor_tensor(
                        scr[:], ht[:], 0.0, ht[:], op0=OP.add, op1=OP.mult,
                        accum_out=sumcols[h][c_o][:, 32 + ti:32 + ti + 1])
                    nc.sync.dma_start(io['hpre'][h, c_o, :, q0:q0 + 512], ht[:])

        # ---- GN stats -> per-channel affine A,B ----
        NG = 32 * NPIX
        for c_o in range(2):
            G = spool.tile([P, 2], F32, tag=f"G{h}{c_o}")
            nc.vector.tensor_reduce(G[:, 0:1], sumcols[h][c_o][:, 0:32], axis=AX.X, op=OP.add)
            nc.vector.tensor_reduce(G[:, 1:2], sumcols[h][c_o][:, 32:64], axis=AX.X, op=OP.add)
            pg = pss.tile([4, 2], F32, tag="pssm")
            nc.tensor.matmul(pg[:], onehot4, G[:], start=True, stop=True)
            gsb = spool.tile([4, 2], F32, tag=f"gsb{h}{c_o}")
            nc.vector.tensor_scalar(gsb[:], pg[:], 1.0 / NG, None, op0=OP.mult)
            mu = gsb[:, 0:1]
            e2 = gsb[:, 1:2]
            var = spool.tile([4, 1], F32, tag=f"var{h}{c_o}")
            nc.vector.scalar_tensor_tensor(var[:], mu, mu, e2, op0=OP.mult, op1=OP.subtract)
            # var now = mu*mu - e2 = -true_var; rsd = 1/sqrt(e2 - mu^2 + eps)
            nvar = spool.tile([4, 1], F32, tag=f"nvar{h}{c_o}")
            nc.vector.tensor_scalar(nvar[:], var[:], -1.0, EPS, op0=OP.mult, op1=OP.add)
            sq = spool.tile([4, 1], F32, tag=f"sq{h}{c_o}")
            nc.scalar.sqrt(sq[:], nvar[:])
            rsd = spool.tile([4, 1], F32, tag=f"rsd{h}{c_o}")
            nc.vector.reciprocal(rsd[:], sq[:])
            pb1 = pss.tile([P, 2], F32, tag="pssm")
            nc.tensor.matmul(pb1[:, 0:1], tcb[:], rsd[:], start=True, stop=True)
            nc.tensor.matmul(pb1[:, 1:2], tcb[:], mu, start=True, stop=True)
            A = spool.tile([P, 1], F32, tag=f"A{h}{c_o}")
            nc.vector.tensor_tensor(A[:], gnt[h][c_o][:, 0:1], pb1[:, 0:1], op=OP.mult)
            B = spool.tile([P, 1], F32, tag=f"B{h}{c_o}")
            nc.vector.scalar_tensor_tensor(B[:], pb1[:, 1:2], A[:], gnt[h][c_o][:, 1:2],
                                           op0=OP.mult, op1=OP.subtract)
            # B = gn_beta - mu*A  (stt: (mu mult A) subtract beta = mu*A - beta -> negate)
            nc.vector.tensor_scalar(B[:], B[:], -1.0, None, op0=OP.mult)
            AB[h][c_o] = (A, B)

        # ---- pass 2: affine + gelu + conv1x1 ----
        if KSTAGE < 2:
            continue
        co = co_heads[h]
        for t in range(32):
            q0 = t * 512
            pl = ps1.tile([2, 512], F32)
            for ci in range(2):
                hp = p2pool.tile([P, 512], F32, tag="hp")
                nc.sync.dma_start(hp[:], io['hpre'][h, ci, :, q0:q0 + 512])
                A, B = AB[h][ci]
                nc.vector.tensor_scalar(hp[:], hp[:], A[:], B[:], op0=OP.mult, op1=OP.add)
                ga = gapool.tile([P, 512], F32R, tag="ga")
                nc.scalar.activation(ga[:], hp[:], AF.Gelu)
                nc.tensor.matmul(pl[0:co, :], w2t[h][ci][:, 0:co], ga[:],
                                 start=(ci == 0), stop=(ci == 1))
            lg = lgpool.tile([2, 512], F32, tag="lg")
            nc.scalar.activation(lg[0:co, :], pl[0:co, :], AF.Identity,
                                 bias=b2t[h][0:co, 0:1], scale=1.0)
            if h == 0:
                nc.sync.dma_start(io['o_dmg'][0:1, q0:q0 + 512], lg[0:1, :])
            else:
                nc.sync.dma_start(io['o_corn'][0:2, q0:q0 + 512], lg[0:2, :])

    # ============ decode (pixel-major [128,128]) ============
    if KSTAGE < 3:
        ctx.close()
        return
    def dtile(tag):
        return dpool.tile([P, HW], F32, tag=tag, name=tag)

    ddmg = dtile("ddmg")
    nc.sync.dma_start(ddmg[:], io['o_dmg'].rearrange("o (p f) -> o p f", f=HW)[0])
    dc0 = dtile("dc0")
    dc1 = dtile("dc1")
    cview = io['o_corn'].rearrange("o (p f) -> o p f", f=HW)
    nc.sync.dma_start(dc0[:], cview[0])
    nc.sync.dma_start(dc1[:], cview[1])

    p_dmg = dtile("p_dmg")
    nc.scalar.activation(p_dmg[:], ddmg[:], AF.Sigmoid)
    s0 = dtile("s0")
    nc.scalar.activation(s0[:], dc0[:], AF.Sigmoid)
    s1 = dtile("s1")
    nc.scalar.activation(s1[:], dc1[:], AF.Sigmoid)
    t1 = dtile("t1")
    nc.vector.tensor_tensor(t1[:], s0[:], s1[:], op=OP.mult)
    sev = [dtile("sev0"), dtile("sev1"), dtile("sev2")]
    nc.vector.tensor_scalar(sev[0][:], s0[:], -1.0, 1.0, op0=OP.mult, op1=OP.add)
    nc.vector.tensor_tensor(sev[1][:], s0[:], t1[:], op=OP.subtract)
    nc.vector.tensor_copy(sev[2][:], t1[:])

    def normalize(chans, tagp):
        for chn in chans:
            nc.vector.tensor_scalar(chn[:], chn[:], 1e-8, None, op0=OP.max)
        den = dpool.tile([P, HW], F32, tag=f"{tagp}den")
        nc.vector.tensor_tensor(den[:], chans[0][:], chans[1][:], op=OP.add)
        for chn in chans[2:]:
            nc.vector.tensor_tensor(den[:], den[:], chn[:], op=OP.add)
        nc.vector.tensor_scalar(den[:], den[:], 1e-8, None, op0=OP.max)
        rd = dpool.tile([P, HW], F32, tag=f"{tagp}rd")
        nc.vector.reciprocal(rd[:], den[:])
        for chn in chans:
            nc.vector.tensor_tensor(chn[:], chn[:], rd[:], op=OP.mult)
        return chans

    sev = normalize(sev, "sv")
    pix = [dtile("pix0")]
    nc.vector.tensor_scalar(pix[0][:], p_dmg[:], -1.0, 1.0, op0=OP.mult, op1=OP.add)
    for i in range(3):
        pt = dtile(f"pix{i + 1}")
        nc.vector.tensor_tensor(pt[:], p_dmg[:], sev[i][:], op=OP.mult)
        pix.append(pt)
    pix = normalize(pix, "px")
    for c in range(4):
        nc.sync.dma_start(io['o_pix'][c:c + 1, :], pix[c][:])

    # argmax(sev)+1 masked by p_dmg >= 0.5
    gt1 = dpool.tile([P, HW], I32, tag="gt1")
    nc.vector.tensor_tensor(gt1[:], sev[1][:], sev[0][:], op=OP.is_gt)
    best = dtile("best")
    nc.vector.tensor_tensor(best[:], sev[0][:], sev[1][:], op=OP.max)
    gt2 = dpool.tile([P, HW], I32, tag="gt2")
    nc.vector.tensor_tensor(gt2[:], sev[2][:], best[:], op=OP.is_gt)
    idx = dtile("idx")
    nc.vector.tensor_copy(idx[:], gt1[:])          # 0/1 as float
    two = dtile("two")
    nc.vector.memset(two[:], 2.0)
    idx2 = dtile("idx2")
    nc.vector.select(idx2[:], gt2[:], two[:], idx[:])
    pge = dtile("pge")
    nc.vector.tensor_scalar(pge[:], p_dmg[:], 0.5, None, op0=OP.is_ge)
    predf = dtile("predf")
    nc.vector.tensor_scalar(predf[:], idx2[:], 1.0, None, op0=OP.add)
    nc.vector.tensor_tensor(predf[:], predf[:], pge[:], op=OP.mult)
    predi = dpool.tile([P, HW], I32, tag="predi")
    nc.vector.tensor_copy(predi[:], predf[:])
    nc.sync.dma_start(io['o_pred'], predi[:])

    # ---------------- blocked [128,512] aggregation ----------------
    if KSTAGE < 4:
        ctx.close()
        return
    m4raw = dpool.tile([P, 512], F32, tag="m4raw")
    mview = io['mrow'].rearrange("o (b f) -> o b f", b=32)
    for c in range(4):
        nc.sync.dma_start(m4raw[32 * c:32 * c + 32, :], mview[0])
    vmask4 = dpool.tile([P, 512], I32, tag="vmask4")
    nc.vector.tensor_scalar(vmask4[:], m4raw[:], 0.5, None, op0=OP.is_gt)
    pix4 = dpool.tile([P, 512], F32, tag="pix4")
    pxv = io['o_pix'].rearrange("c (b f) -> c b f", b=32)
    for c in range(4):
        nc.sync.dma_start(pix4[32 * c:32 * c + 32, :], pxv[c])
    neg1 = dpool.tile([P, 512], F32, tag="neg1")
    nc.vector.memset(neg1[:], -1.0)
    M4 = dpool.tile([P, 512], F32, tag="M4")
    nc.vector.select(M4[:], vmask4[:], pix4[:], neg1[:])

    scr = dpool.tile([P, 512], F32, tag="bscr")
    m_part = spool.tile([P, 1], F32, tag="m_part")
    nc.vector.tensor_scalar(scr[:], M4[:], 0.0, 0.0, op0=OP.max, op1=OP.add,
                            accum_out=m_part[:])
    cnt_part = spool.tile([P, 1], F32, tag="cnt_part")
    nc.vector.tensor_scalar(scr[:], M4[:], 0.0, 0.0, op0=OP.is_gt, op1=OP.add,
                            accum_out=cnt_part[:])

    # cnt, k on [1,1]
    pc = pss.tile([1, 1], F32, tag="pssm")
    nc.tensor.matmul(pc[:], ones_col, cnt_part[:], start=True, stop=True)
    cnt = spool.tile([1, 1], F32, tag="cnt")
    nc.vector.tensor_scalar(cnt[:], pc[:], 0.25, None, op0=OP.mult)
    kc = spool.tile([1, 1], F32, tag="kc")
    nc.vector.tensor_scalar(kc[:], cnt[:], TOPK_RATIO, None, op0=OP.mult)
    ki = spool.tile([1, 1], I32, tag="ki")
    nc.vector.tensor_copy(ki[:], kc[:])            # rint
    kf = spool.tile([1, 1], F32, tag="kf")
    nc.vector.tensor_copy(kf[:], ki[:])
    mx1 = spool.tile([1, 1], F32, tag="mx1")
    nc.vector.tensor_scalar(mx1[:], cnt[:], 1.0, None, op0=OP.max)
    kk = spool.tile([1, 1], F32, tag="kk")
    nc.vector.tensor_scalar(kk[:], kf[:], 1.0, None, op0=OP.max)
    nc.vector.tensor_tensor(kk[:], kk[:], mx1[:], op=OP.min)
    pk = pss.tile([P, 1], F32, tag="pssm")
    nc.tensor.matmul(pk[:], ones_row, kk[:], start=True, stop=True)
    kbc = spool.tile([P, 1], F32, tag="kbc")
    nc.vector.tensor_copy(kbc[:], pk[:])

    # binary search
    lo = spool.tile([P, 1], F32, tag="bs_lo0")
    hi = spool.tile([P, 1], F32, tag="bs_hi0")
    nc.vector.memset(lo[:], 0.0)
    nc.vector.memset(hi[:], 1.0)
    for it in range(BSEARCH_ITERS):
        mid = spool.tile([P, 1], F32, tag=f"bs_mid{it % 2}")
        nc.vector.tensor_tensor(mid[:], lo[:], hi[:], op=OP.add)
        nc.vector.tensor_scalar(mid[:], mid[:], 0.5, None, op0=OP.mult)
        cg = spool.tile([P, 1], F32, tag=f"bs_cg{it % 2}")
        nc.vector.tensor_scalar(scr[:], M4[:], mid[:], 0.0, op0=OP.is_gt, op1=OP.add,
                                accum_out=cg[:])
        psb = pss.tile([P, 1], F32, tag="pssm")
        nc.tensor.matmul(psb[:], blockones, cg[:], start=True, stop=True)
        ctot = spool.tile([P, 1], F32, tag=f"bs_ct{it % 2}")
        nc.vector.tensor_copy(ctot[:], psb[:])
        ge = spool.tile([P, 1], I32, tag=f"bs_ge{it % 2}")
        nc.vector.tensor_tensor(ge[:], ctot[:], kbc[:], op=OP.is_ge)
        lo2 = spool.tile([P, 1], F32, tag=f"bs_lo{1 + it % 2}a")
        hi2 = spool.tile([P, 1], F32, tag=f"bs_hi{1 + it % 2}a")
        nc.vector.select(lo2[:], ge[:], mid[:], lo[:])
        nc.vector.select(hi2[:], ge[:], hi[:], mid[:])
        lo, hi = lo2, hi2

    s_part = spool.tile([P, 1], F32, tag="s_part")
    nc.vector.scalar_tensor_tensor(scr[:], M4[:], hi[:], M4[:], op0=OP.is_gt,
                                   op1=OP.mult, accum_out=s_part[:])
    cgt_part = spool.tile([P, 1], F32, tag="cgt_part")
    nc.vector.tensor_scalar(scr[:], M4[:], hi[:], 0.0, op0=OP.is_gt, op1=OP.add,
                            accum_out=cgt_part[:])

    # assembly: Q [128,16] -> row16 [1,16]
    Q = dpool.tile([P, 16], F32, tag="Q")
    nc.vector.tensor_scalar(Q[:, 0:4], onehot4, m_part[:], None, op0=OP.mult)
    nc.vector.tensor_scalar(Q[:, 4:8], onehot4, s_part[:], None, op0=OP.mult)
    nc.vector.tensor_scalar(Q[:, 8:12], onehot4, cgt_part[:], None, op0=OP.mult)
    hi32 = spool.tile([P, 1], F32, tag="hi32")
    nc.vector.tensor_scalar(hi32[:], hi[:], 1.0 / 32.0, None, op0=OP.mult)
    nc.vector.tensor_scalar(Q[:, 12:16], onehot4, hi32[:], None, op0=OP.mult)
    pq = pss.tile([1, 16], F32, tag="pssm")
    nc.tensor.matmul(pq[:], ones_col, Q[:], start=True, stop=True)
    row16 = spool.tile([1, 16], F32, tag="row16")
    nc.vector.tensor_copy(row16[:], pq[:])

    # final [1,4] math
    unif4 = spool.tile([1, 4], F32, tag="unif4")
    nc.vector.memset(unif4[:], 0.25)
    z4 = spool.tile([1, 4], F32, tag="z4")
    nc.vector.memset(z4[:], 0.0)
    m0 = spool.tile([1, 4], I32, tag="m0")
    nc.vector.tensor_scalar(m0[:], z4[:], cnt[:], 0.5, op0=OP.add, op1=OP.is_lt)

    rdm = spool.tile([1, 1], F32, tag="rdm")
    nc.vector.reciprocal(rdm[:], mx1[:])
    mean_pre = spool.tile([1, 4], F32, tag="mean_pre")
    nc.vector.tensor_scalar(mean_pre[:], row16[:, 0:4], rdm[:], None, op0=OP.mult)
    mean_sel = spool.tile([1, 4], F32, tag="mean_sel")
    nc.vector.select(mean_sel[:], m0[:], unif4[:], mean_pre[:])

    def norm4(x, tagp):
        nm = spool.tile([1, 4], F32, tag=f"{tagp}nm")
        nc.vector.tensor_scalar(nm[:], x[:], 1e-8, None, op0=OP.max)
        s = spool.tile([1, 1], F32, tag=f"{tagp}s")
        nc.vector.tensor_reduce(s[:], nm[:], axis=AX.XYZW, op=OP.add)
        nc.vector.tensor_scalar(s[:], s[:], 1e-8, None, op0=OP.max)
        rs = spool.tile([1, 1], F32, tag=f"{tagp}rs")
        nc.vector.reciprocal(rs[:], s[:])
        o = spool.tile([1, 4], F32, tag=f"{tagp}o")
        nc.vector.tensor_scalar(o[:], nm[:], rs[:], None, op0=OP.mult)
        return o

    mean_p = norm4(mean_sel, "mp")
    nc.sync.dma_start(io['o_mean'], mean_p[:])

    km1 = spool.tile([1, 4], F32, tag="km1")
    nc.vector.tensor_scalar(km1[:], row16[:, 8:12], kk[:], None, op0=OP.subtract)
    u4 = spool.tile([1, 4], F32, tag="u4")
    nc.vector.tensor_tensor(u4[:], km1[:], row16[:, 12:16], op=OP.mult)
    ts4 = spool.tile([1, 4], F32, tag="ts4")
    nc.vector.tensor_tensor(ts4[:], row16[:, 4:8], u4[:], op=OP.subtract)
    rk = spool.tile([1, 1], F32, tag="rk")
    nc.vector.reciprocal(rk[:], kk[:])
    tp_pre = spool.tile([1, 4], F32, tag="tp_pre")
    nc.vector.tensor_scalar(tp_pre[:], ts4[:], rk[:], None, op0=OP.mult)
    tp_sel = spool.tile([1, 4], F32, tag="tp_sel")
    nc.vector.select(tp_sel[:], m0[:], unif4[:], tp_pre[:])
    topk_p = norm4(tp_sel, "tp")
    nc.sync.dma_start(io['o_topk'], topk_p[:])

    a1 = spool.tile([1, 4], F32, tag="a1")
    nc.vector.tensor_scalar(a1[:], mean_p[:], MEAN_W, None, op0=OP.mult)
    a2 = spool.tile([1, 4], F32, tag="a2")
    nc.vector.scalar_tensor_tensor(a2[:], topk_p[:], TOPK_W, a1[:], op0=OP.mult, op1=OP.add)
    agg = norm4(a2, "ag")
    nc.sync.dma_start(io['o_agg'], agg[:])

    amx = spool.tile([1, 1], F32, tag="amx")
    nc.vector.tensor_reduce(amx[:], agg[:], axis=AX.XYZW, op=OP.max)
    eq = spool.tile([1, 4], I32, tag="eq")
    nc.vector.tensor_scalar(eq[:], agg[:], amx[:], None, op0=OP.is_equal)
    big = spool.tile([1, 4], F32, tag="big")
    nc.vector.memset(big[:], 99.0)
    cand = spool.tile([1, 4], F32, tag="cand")
    nc.vector.select(cand[:], eq[:], iota4, big[:])
    albl = spool.tile([1, 1], F32, tag="albl")
    nc.vector.tensor_reduce(albl[:], cand[:], axis=AX.XYZW, op=OP.min)
    albi = spool.tile([1, 1], I32, tag="albi")
    nc.vector.tensor_copy(albi[:], albl[:])
    nc.sync.dma_start(io['o_albl'], albi[:])

    ctx.close()


def _build():
    nc = bacc.Bacc("TRN2", target_bir_lowering=False, debug=False,
                   num_devices=NCORES)
    io = {}
    io['x'] = nc.dram_tensor("x", [CI, NPIX], F32, kind="ExternalInput").ap()
    io['mrow'] = nc.dram_tensor("mrow", [1, NPIX], F32, kind="ExternalInput").ap()
    io['w1'] = nc.dram_tensor("w1", [2, 2, 128, 9 * 256], F32, kind="ExternalInput").ap()
    io['w2'] = nc.dram_tensor("w2", [2, 2, 128, 2], F32, kind="ExternalInput").ap()
    io['gn'] = nc.dram_tensor("gn", [2, 2, 128, 2], F32, kind="ExternalInput").ap()
    io['b2'] = nc.dram_tensor("b2", [2, 2, 1], F32, kind="ExternalInput").ap()
    io['cst'] = nc.dram_tensor("cst", [128, 133], F32, kind="ExternalInput").ap()
    io['crow'] = nc.dram_tensor("crow", [1, 136], F32, kind="ExternalInput").ap()
    io['cstb'] = nc.dram_tensor("cstb", [4, 128], F32, kind="ExternalInput").ap()
    io['zeros'] = nc.dram_tensor("zeros", [128, 130], F32, kind="ExternalInput").ap()
    io['hpre'] = nc.dram_tensor("hpre", [2, 2, 128, NPIX], F32, kind="Internal").ap()
    io['o_dmg'] = nc.dram_tensor("o_dmg", [1, NPIX], F32, kind="ExternalOutput").ap()
    io['o_corn'] = nc.dram_tensor("o_corn", [2, NPIX], F32, kind="ExternalOutput").ap()
    io['o_pix'] = nc.dram_tensor("o_pix", [4, NPIX], F32, kind="ExternalOutput").ap()
    io['o_pred'] = nc.dram_tensor("o_pred", [128, HW], I32, kind="ExternalOutput").ap()
    io['o_mean'] = nc.dram_tensor("o_mean", [1, 4], F32, kind="ExternalOutput").ap()
    io['o_topk'] = nc.dram_tensor("o_topk", [1, 4], F32, kind="ExternalOutput").ap()
    io['o_agg'] = nc.dram_tensor("o_agg", [1, 4], F32, kind="ExternalOutput").ap()
    io['o_albl'] = nc.dram_tensor("o_albl", [1, 1], I32, kind="ExternalOutput").ap()

    with tile.TileContext(nc) as tc:
        _emit(nc, tc, io)
    nc.compile()
    return nc


def _host_consts():
    CST = np.zeros((128, 133), np.float32)
    for g in range(4):
        CST[32 * g:32 * g + 32, 32 * g:32 * g + 32] = 1.0
    CST[:, 128] = 1.0
    for g in range(4):
        CST[32 * g:32 * g + 32, 129 + g] = 1.0
    CROW = np.zeros((1, 136), np.float32)
    CROW[0, :128] = 1.0
    CROW[0, 128:132] = [0, 1, 2, 3]
    CSTB = np.zeros((4, 128), np.float32)
    for g in range(4):
        CSTB[g, 32 * g:32 * g + 32] = 1.0
    return CST, CROW, CSTB


def _prep_weights(dw1, dgs, dgb, dw2, db2, sw1, sgs, sgb, sw2, sb2):
    W1 = np.zeros((2, 2, 128, 9 * 256), np.float32)
    for h, w in enumerate([dw1, sw1]):
        # [co,ci,ky,kx] -> [ky*kx, ci, co] -> [2,128, 9, 256]
        t = np.ascontiguousarray(np.transpose(np.asarray(w, np.float32), (2, 3, 1, 0)))
        t = t.reshape(9, 256, 256)                     # [tap, ci, co]
        t = np.transpose(t, (1, 0, 2)).reshape(2, 128, 9 * 256)
        W1[h] = t
    W2 = np.zeros((2, 2, 128, 2), np.float32)
    for h, w in enumerate([dw2, sw2]):
        co = w.shape[0]
        t = np.asarray(w, np.float32).reshape(co, 256)  # [co, ci]
        W2[h, :, :, :co] = t.T.reshape(2, 128, co)
    GN = np.zeros((2, 2, 128, 2), np.float32)
    for h, (gs, gb) in enumerate([(dgs, dgb), (sgs, sgb)]):
        GN[h, :, :, 0] = np.asarray(gs, np.float32).reshape(2, 128)
        GN[h, :, :, 1] = np.asarray(gb, np.float32).reshape(2, 128)
    B2 = np.zeros((2, 2, 1), np.float32)
    B2[0, 0, 0] = np.asarray(db2, np.float32)[0]
    B2[1, :, 0] = np.asarray(sb2, np.float32)
    return W1, W2, GN, B2


def kernel(feature_map, target_mask, dw1, dgs, dgb, dw2, db2,
           sw1, sgs, sgb, sw2, sb2):
    if 'nc' not in _cache:
        _cache['nc'] = _build()
    nc = _cache['nc']

    B = feature_map.shape[0]
    assert B == NCORES
    W1, W2, GN, B2 = _prep_weights(dw1, dgs, dgb, dw2, db2, sw1, sgs, sgb, sw2, sb2)
    CST, CROW, CSTB = _host_consts()
    fm = np.ascontiguousarray(np.asarray(feature_map, np.float32))
    tm = np.ascontiguousarray(np.asarray(target_mask, np.float32))

    in_maps = []
    for i in range(NCORES):
        in_maps.append({
            'x': fm[i].reshape(CI, NPIX),
            'mrow': tm[i].reshape(1, NPIX),
            'w1': W1, 'w2': W2, 'gn': GN, 'b2': B2,
            'cst': CST, 'crow': CROW, 'cstb': CSTB,
            'zeros': np.zeros((128, 130), np.float32),
        })
    res = run_bass_kernel_spmd(nc, in_maps, core_ids=list(range(NCORES)))
    rs = res.results

    dmg = np.stack([rs[i]['o_dmg'].reshape(1, HW, HW) for i in range(B)])
    corn = np.stack([rs[i]['o_corn'].reshape(2, HW, HW) for i in range(B)])
    pix = np.stack([rs[i]['o_pix'].reshape(4, HW, HW) for i in range(B)])
    pred = np.stack([rs[i]['o_pred'].reshape(HW, HW) for i in range(B)]).astype(np.int32)
    mean_p = np.stack([rs[i]['o_mean'].reshape(4) for i in range(B)])
    topk_p = np.stack([rs[i]['o_topk'].reshape(4) for i in range(B)])
    agg = np.stack([rs[i]['o_agg'].reshape(4) for i in range(B)])
    albl = np.stack([rs[i]['o_albl'].reshape(()) for i in range(B)]).astype(np.int32)
    return (dmg, corn, pix, pred, mean_p, topk_p, agg, albl)
